# revision 22
# baseline (speedup 1.0000x reference)
"""KAN layer Trainium2 kernel.

Math: out[b,o] = sum_{i,g} exp(-|tanh(x[b,i]) - grid[g]| * s[o,i]) * w[o,i,g]

For t = tanh(x) in grid interval v (grid[v] <= t < grid[v+1]):
    f_{o,i}(t) = P_v * e^{-s t} + S_{v+1} * e^{s t}
with P_v = sum_{j<=v} w_j e^{s g_j}, S_{v+1} = sum_{j>v} w_j e^{-s g_j}.
Each piece is expanded in a degree-(NCHEB-1) Chebyshev basis of the
interval-local coordinate d = 7t + 6 - 2v, giving

    out[b,o] = sum_{i,v,c} mask_v(t[b,i]) * T_c(d[b,i]) * D[(v,c,i), o]

one (B x NV*NCHEB*I) @ (NV*NCHEB*I x O) matmul. Sharding: 2D over
(batch quarters x i-halves) — core cid handles batch rows
[(cid%4)*256, ...) and input features [(cid//4)*128, ...); the host sums
the two partial outputs per batch quarter (part of unsharding). This
halves the per-core D stream (1.37 MB bf16), which is the wall: HBM per
core is ~358 GB/s and everything else overlaps under it.

Device structure per core (256 batch rows, 128 i's on partitions):
- interval index k = round(3.5 t + 3) via the fp32 +M/-M trick
  (M = 1.5*2^23; all immediates are bf16-exact — the tensor_scalar
  immediate path quantizes, e.g. 12582911.5 behaves as M).
- masks (k==v) double as the c=0 matmul lhsT; c>=1 lhsT are plain
  bf16 tensor_tensor products mask*T_c (2x DVE mode, no broadcasts).
- both matmul operands bf16 (fp32 matmuls issue as 2 HI/LO instructions
  with slow weight loads; bf16 enables FWL); 42 accumulating matmuls
  into two PSUM tiles (batch halves).
- D issued first as 4 large sync-ring DMAs (many small DMAs measured
  ~171 GB/s; big chunks ~310 GB/s); x arrives in parallel on the GpSimd
  SWDGE ring; dummy matmuls open the PE clock gate meanwhile.
"""

import numpy as np
import ml_dtypes

B, I, O, G = 1024, 256, 256, 8
NV = G - 1            # 7 intervals
NCHEB = 3             # degree-2 Chebyshev per interval
N_CORES = 8
PB = 4                # batch shards
PK = 2                # i shards
BSH = B // PB         # 256 batch rows per core
ISH = I // PK         # 128 i's per core (one partition set)
NWARM = 12            # dummy matmuls to open the PE clock gate
VCHUNKS = ((0, 2), (2, 4), (4, 6), (6, 7))  # D chunk v-ranges
SCQ = 1.4140625       # bf16(sqrt(2)): q2 basis = (SCQ*d)^2 = Square(SCQ*d)

_CACHE = {}


def _precompute_dmat(spline_weight, spline_scaler, grid):
    """D as (PK, 128, NV*NCHEB*O) bf16: [ih, i, ((v*NCHEB + c)*O + o)]."""
    w = spline_weight.astype(np.float64)          # (O, I, G)
    s = spline_scaler.astype(np.float64)          # (O, I)
    g = grid.astype(np.float64)                   # (G,)
    OI = O * I

    Eg = np.exp(g[None, None, :] * s[:, :, None])             # (O,I,G)
    P = np.cumsum(w * Eg, axis=2)                              # prefix_j<=v
    S = np.cumsum((w / Eg)[:, :, ::-1], axis=2)[:, :, ::-1]    # suffix_j>=v

    h = 1.0 / NV
    centers = -1.0 + (2 * np.arange(NV) + 1) * h

    # Least-squares fit of each interval piece in the device-exact basis
    # {1, d, (SC*d)^2} at Chebyshev nodes (SC = bf16(sqrt(2)); the device
    # computes q2 = Square(SC*d) on the scalar engine).
    M = 32
    nodes = np.cos(np.pi * (np.arange(M) + 0.5) / M)
    A = np.stack([np.ones(M), nodes, (SCQ * nodes) ** 2], axis=1)[:, :NCHEB]
    pinv = np.linalg.pinv(A)                                    # (NCHEB, M)
    sf = s.reshape(-1)                                          # (O*I,)
    Fm = np.exp(-np.outer(sf * h, nodes))                       # e^{-s h d_m}
    Pf = P.reshape(OI, G)
    Sf = S.reshape(OI, G)
    D = np.empty((NV, NCHEB, OI))
    for v in range(NV):
        em = np.exp(-sf * centers[v])
        fv = (Pf[:, v] * em)[:, None] * Fm + (Sf[:, v + 1] / em)[:, None] / Fm
        D[v] = pinv @ fv.T                                      # (NCHEB, O*I)
    # (NV, NCHEB, O, I) -> (PK, 128i, NV*NCHEB*O)
    Dm = D.reshape(NV, NCHEB, O, PK, ISH).transpose(3, 4, 0, 1, 2)
    Dm = Dm.reshape(PK, ISH, NV * NCHEB * O)
    return np.ascontiguousarray(Dm).astype(ml_dtypes.bfloat16)


def _build_module():
    import concourse.bacc as bacc
    import concourse.bass as bass
    import concourse.mybir as mybir
    import concourse.tile as tile

    f32 = mybir.dt.float32
    bf16 = mybir.dt.bfloat16
    AF = mybir.ActivationFunctionType
    ALU = mybir.AluOpType

    nc = bacc.Bacc("TRN2", target_bir_lowering=False, debug=False,
                   num_devices=N_CORES)

    xT = nc.dram_tensor("xt", [ISH, BSH], f32, kind="ExternalInput")
    dmat = nc.dram_tensor("dmat", [ISH, NV * NCHEB * O], bf16,
                          kind="ExternalInput")
    out_d = nc.dram_tensor("out", [BSH // 2, 2 * O], f32, kind="ExternalOutput")

    with tile.TileContext(nc) as tc:
        with (
            tc.tile_pool(name="keep", bufs=1) as keep,
            tc.tile_pool(name="dpool", bufs=1) as dpool,
            tc.tile_pool(name="prod", bufs=1) as prod,
            tc.tile_pool(name="psum", bufs=1, space=bass.MemorySpace.PSUM) as ppool,
        ):
            # x first on the sync HWDGE ring (it heads the serial basis
            # chain), then the D chunks in matmul consumption order.
            xsb = keep.tile([ISH, BSH], f32, tag="x", name="x")
            nc.sync.dma_start(xsb[:], xT[:])
            dchunk = [None] * len(VCHUNKS)
            for q, (v0, v1) in enumerate(VCHUNKS):
                cw = (v1 - v0) * NCHEB * O
                c0 = v0 * NCHEB * O
                dt_ = dpool.tile([ISH, cw], bf16, tag=f"d{q}", name=f"d{q}")
                nc.sync.dma_start(dt_[:], dmat[:, c0:c0 + cw])
                dchunk[q] = dt_

            # Dummy matmuls to open the PE clock gate during basis compute.
            wz = keep.tile([128, 512], bf16, tag="warm", name="warm")
            nc.vector.memset(wz[:], 0.0)
            wps = ppool.tile([128, 512], f32, tag="wps", name="wps")
            for _ in range(NWARM):
                nc.tensor.matmul(wps[:], wz[:, :128], wz[:],
                                 start=True, stop=True)

            # Basis: t = tanh(x); k = round(3.5t+3) (+M/-M round trick, RTE
            # ties at grid points are harmless); d = (7t+6) - 2k in [-1,1];
            # tcat = [d | T2] with T2 = 2d^2 - 1.
            MAGIC = 12582912.0  # 1.5 * 2^23
            t = keep.tile([ISH, BSH], f32, tag="t", name="t")
            nc.scalar.activation(t[:], xsb[:], AF.Tanh)
            ua = keep.tile([ISH, BSH], f32, tag="ua", name="ua")
            nc.vector.tensor_scalar(ua[:], t[:], 3.5, 3.0, ALU.mult, ALU.add)
            r1 = keep.tile([ISH, BSH], f32, tag="r1", name="r1")
            nc.vector.tensor_scalar(r1[:], ua[:], MAGIC, None, ALU.add)
            kf = keep.tile([ISH, BSH], f32, tag="kf", name="kf")
            nc.vector.tensor_scalar(kf[:], r1[:], MAGIC, None, ALU.subtract)
            kb = keep.tile([ISH, BSH], bf16, tag="kb", name="kb")
            nc.vector.tensor_scalar(kb[:], r1[:], MAGIC, None, ALU.subtract)
            u7 = keep.tile([ISH, BSH], f32, tag="u7", name="u7")
            nc.vector.tensor_scalar(u7[:], t[:], 7.0, 6.0, ALU.mult, ALU.add)
            db = keep.tile([ISH, BSH], bf16, tag="db", name="db")
            nc.vector.scalar_tensor_tensor(db[:], kf[:], -2.0, u7[:],
                                           ALU.mult, ALU.add)
            q2 = keep.tile([ISH, BSH], bf16, tag="q2", name="q2")
            nc.scalar.activation(q2[:], db[:], AF.Square, scale=SCQ)
            cheb = [None, db, q2]

            # Masks (= c=0 lhsT) and per-c products (c=1 on DVE, c=2 on
            # GpSimd, which supports plain TT). Matmuls run batch-half-major
            # so the first half's PSUM drains (copy + DMA out) while the
            # second half's matmuls still stream.
            accs = [ppool.tile([BSH // 2, O], f32, tag=f"acc{bh}",
                               name=f"acc{bh}") for bh in range(2)]
            osb = keep.tile([BSH // 2, 2 * O], f32, tag="o", name="o")
            masks = [None] * NV
            for v in range(NV):
                mv = keep.tile([ISH, BSH], bf16, tag=f"m{v}", name=f"m{v}")
                nc.vector.tensor_scalar(mv[:], kb[:], float(v), None,
                                        ALU.is_equal)
                masks[v] = mv
            prods = [[None] * NCHEB for _ in range(NV)]
            n_mm = NV * NCHEB
            idx = 0
            for v in range(NV):
                for c in range(1, NCHEB):
                    pc_ = prod.tile([ISH, BSH], bf16, tag=f"p{v}_{c}",
                                    name=f"p{v}_{c}")
                    nc.vector.tensor_tensor(
                        pc_[:], masks[v][:], cheb[c][:], ALU.mult)
                    prods[v][c] = pc_
                q = next(i for i, (v0, v1) in enumerate(VCHUNKS)
                         if v0 <= v < v1)
                voff = (v - VCHUNKS[q][0]) * NCHEB * O
                for c in range(NCHEB):
                    lt = masks[v][:] if c == 0 else prods[v][c][:]
                    for bh in range(2):
                        nc.tensor.matmul(
                            accs[bh][:], lt[:, bh * 128:(bh + 1) * 128],
                            dchunk[q][:, voff + c * O:voff + (c + 1) * O],
                            start=(idx == 0), stop=(idx == n_mm - 1))
                    idx += 1
            for bh in range(2):
                nc.scalar.copy(osb[:, bh * O:(bh + 1) * O], accs[bh][:])
                nc.sync.dma_start(out_d[:, bh * O:(bh + 1) * O],
                                  osb[:, bh * O:(bh + 1) * O])

    nc.compile()
    return nc


def kernel(x, spline_weight, spline_scaler, grid):
    from concourse import bass_utils

    x = np.asarray(x, dtype=np.float32)
    Dm = _precompute_dmat(np.asarray(spline_weight), np.asarray(spline_scaler),
                          np.asarray(grid))

    if "nc" not in _CACHE:
        _CACHE["nc"] = _build_module()
    nc = _CACHE["nc"]

    in_maps = []
    for cid in range(N_CORES):
        bq, ih = cid % PB, cid // PB
        xs = x[bq * BSH:(bq + 1) * BSH, ih * ISH:(ih + 1) * ISH]   # (BSH, ISH)
        in_maps.append({"xt": np.ascontiguousarray(xs.T), "dmat": Dm[ih]})

    import os
    trace = bool(int(os.environ.get("KAN_TRACE", "0")))
    kw = {}
    if trace:
        tdir = os.environ.get("KAN_TRACE_DIR") or None
        kw = dict(trace=True, tmpdir=tdir)
    res = bass_utils.run_bass_kernel_spmd(nc, in_maps,
                                          core_ids=list(range(N_CORES)), **kw)
    _CACHE["last_result"] = res
    out = np.empty((B, O), dtype=np.float32)
    for bq in range(PB):
        part = (res.results[bq]["out"].astype(np.float32)
                + res.results[bq + PB]["out"].astype(np.float32))
        out[bq * BSH:bq * BSH + BSH // 2] = part[:, :O]
        out[bq * BSH + BSH // 2:(bq + 1) * BSH] = part[:, O:]
    return out
